# revision 2
# baseline (speedup 1.0000x reference)
"""GNN message-passing kernel V2 for 8 TRN2 NeuronCores.

V2 changes vs the staged baseline:
- The per-layer m-table (m = h @ W_neigh, all 50176 padded rows) lives in
  SBUF (12.8 MB, token layout: row r -> partition r%128, rank r//128), not
  HBM.  Exchange broadcasts write straight into the table region of the
  sending core; the XOR slot->sender scramble is folded into the per-core
  precomputed gather indices, so there is no staging buffer, no reorder
  pass, and no register arithmetic.
- Messages are fetched with SBUF-source transpose-mode dma_gather (fabric
  bandwidth instead of HBM random-256B reads), then transposed back to
  edge-major with PE transposes batched 8 tiles per PSUM bank, copy-back
  alternating between ACT and DVE.
- Segment sums via PE indicator matmuls as before; agg stored fp16.
"""
import numpy as np

import concourse.bacc as bacc
import concourse.bass as bass
import concourse.mybir as mybir
from concourse.tile import TileContext
from concourse.masks import make_identity
from concourse.tile_rust import add_dep_helper

N_NODES = 50000
N_EDGES = 800000
IN_DIM = 128
LAT = 128
OUT_DIM = 64

NCORE = 8
NPC_REAL = N_NODES // NCORE          # 6250
NBLK = 49                            # blocks of 128 per core
NPC = NBLK * 128                     # 6272 padded rows per core region
NTOT = NPC * NCORE                   # 50176 table rows
NRANK = NTOT // 128                  # 392 ranks of the SBUF table
LO_RANKS = 256                       # rows < 32768 (int16 window split)
LO_ROWS = LO_RANKS * 128
CH_T = 16                            # tiles per gather chunk

# HW slot -> delivered tpb delta for remote_dma_broadcast with
# rdests[k]=(0,k): receiver r's slot-k data comes from sender r^DELTA[k]
# (empirical, validated by the baseline kernel on hardware).  The CoreSim
# interpreter instead delivers r = s^k; sim checks monkeypatch this to
# identity.
DELTA = [0, 1, 2, 3, 6, 7, 4, 5]
DEBUG_MTAB = False
NO_GATE = False
NO_PRECRIT_DEPS = False

F16 = mybir.dt.float16
F32 = mybir.dt.float32
I16 = mybir.dt.int16


def preprocess(features, edge_list):
    src = edge_list[:, 0].astype(np.int64)
    dst = edge_list[:, 1].astype(np.int64)
    cnt = np.bincount(src, minlength=N_NODES).astype(np.float32)
    inv_deg_full = (1.0 / np.maximum(cnt, 1.0)).astype(np.float32)

    owner = dst // NPC_REAL
    drel = dst % NPC_REAL
    souner = src // NPC_REAL
    srel_all = src % NPC_REAL
    blk_all = srel_all // 128

    # Per-receiver table position of a dst row: region slot k holds the
    # rows of sender r ^ DELTA[k], so owner c sits in slot DELTA[r ^ c].
    delta = np.asarray(DELTA, np.int64)
    # tpos[r, e] for receiving core r — but edges are partitioned by the
    # SRC owner (aggregation target), so each edge only matters on core
    # souner[e]; compute tpos under that core's view.
    slot = delta[souner ^ owner]
    tok = slot * NPC + drel
    # The SBUF table (token v -> partition v%128, rank v//128) is dumped
    # contiguously per partition into the HBM table, whose row index is
    # therefore idx' = (v%128)*NRANK + v//128.  Gathers read the HBM table.
    tpos = (tok % 128) * NRANK + tok // 128

    order = np.lexsort((tpos, blk_all, souner))
    so, sb = souner[order], blk_all[order]
    sw = (tpos[order] >= LO_ROWS).astype(np.int64)
    st, sr = tpos[order], srel_all[order]
    key = ((so * NBLK) + sb) * 2 + sw
    bounds = np.searchsorted(key, np.arange(NCORE * NBLK * 2 + 1))
    per = {}
    for c in range(NCORE):
        for b in range(NBLK):
            for w in range(2):
                k = (c * NBLK + b) * 2 + w
                lo, hi = bounds[k], bounds[k + 1]
                per[(c, b, w)] = (st[lo:hi], sr[lo:hi])

    T = np.zeros((NBLK, 2), np.int64)
    for b in range(NBLK):
        for w in range(2):
            mx = max(len(per[(c, b, w)][0]) for c in range(NCORE))
            T[b, w] = max(1, -(-mx // 128))

    tiles = []
    for w in range(2):
        for b in range(NBLK):
            for i in range(T[b, w]):
                tiles.append((b, w, i))
    NT = len(tiles)

    chunks = []
    i = 0
    while i < NT:
        w = tiles[i][1]
        j = i
        while j < NT and tiles[j][1] == w and j - i < CH_T:
            j += 1
        chunks.append((i, j - i, w))
        i = j

    gidx = np.zeros((NCORE, 128, NT * 8), np.int16)
    srcrel = np.full((NCORE, 128, NT), -1.0, np.float16)
    for c in range(NCORE):
        for t, (b, w, i) in enumerate(tiles):
            pos, rel = per[(c, b, w)]
            seg = pos[i * 128:(i + 1) * 128]
            relseg = rel[i * 128:(i + 1) * 128]
            n = len(seg)
            idx = np.zeros(128, np.int64)
            idx[:n] = seg - (LO_ROWS if w else 0)
            col = np.full(128, -1.0, np.float32)
            col[:n] = relseg - b * 128
            a = idx.astype(np.int16).reshape(8, 16).T
            gidx[c, :, t * 8:(t + 1) * 8] = np.tile(a, (8, 1))
            srcrel[c, :, t] = col.astype(np.float16)

    featT = np.zeros((NCORE, 128, NPC), np.float32)
    invd = np.ones((NCORE, 128, NBLK), np.float32)
    for c in range(NCORE):
        f = features[c * NPC_REAL:(c + 1) * NPC_REAL]
        featT[c, :, :NPC_REAL] = f.T
        col = np.ones(NPC, np.float32)
        col[:NPC_REAL] = inv_deg_full[c * NPC_REAL:(c + 1) * NPC_REAL]
        invd[c] = col.reshape(NBLK, 128).T

    meta = dict(tiles=tiles, chunks=chunks, T=T, NT=NT)
    return meta, featT, invd, gidx, srcrel


def build(meta):
    tiles, chunks, T, NT = meta["tiles"], meta["chunks"], meta["T"], meta["NT"]
    nc = bacc.Bacc("TRN2", target_bir_lowering=False, debug=False,
                   enable_asserts=True, num_devices=NCORE, num_swdge_queues=4,
                   dynamic_dma_scratch_size=24576)

    featT_d = nc.dram_tensor("featT", [128, NPC], F32, kind="ExternalInput")
    wenc_d = nc.dram_tensor("wenc", [IN_DIM, LAT], F32, kind="ExternalInput")
    benc_d = nc.dram_tensor("benc", [1, LAT], F32, kind="ExternalInput")
    ws_d = [nc.dram_tensor(f"ws{l}", [LAT, LAT], F16, kind="ExternalInput") for l in range(2)]
    wn_d = [nc.dram_tensor(f"wn{l}", [LAT, LAT], F16, kind="ExternalInput") for l in range(2)]
    bc_d = [nc.dram_tensor(f"bc{l}", [1, LAT], F16, kind="ExternalInput") for l in range(2)]
    wout_d = nc.dram_tensor("wout", [LAT, OUT_DIM], F16, kind="ExternalInput")
    bout_d = nc.dram_tensor("bout", [1, OUT_DIM], F16, kind="ExternalInput")
    invd_d = nc.dram_tensor("invd", [128, NBLK], F32, kind="ExternalInput")
    dbg_d = None
    if DEBUG_MTAB:
        dbg_d = nc.dram_tensor("mtab_dbg", [128, NRANK * 128], F16,
                               kind="ExternalOutput")
    gidx_d = nc.dram_tensor("gidx", [128, NT * 8], I16, kind="ExternalInput")
    srcrel_d = nc.dram_tensor("srcrel", [128, NT], F16, kind="ExternalInput")
    out_d = nc.dram_tensor("outp", [NPC, OUT_DIM], F32, kind="ExternalOutput")
    table_d = nc.dram_tensor("table", [NTOT, LAT], F16, kind="Internal")

    with TileContext(nc) as tc:
        with tc.tile_pool(name="consts", bufs=1) as cpool, \
             tc.tile_pool(name="mtabp", bufs=1) as mpool_tab, \
             tc.tile_pool(name="agg", bufs=1) as apool, \
             tc.tile_pool(name="ht", bufs=1) as hpool, \
             tc.tile_pool(name="stage", bufs=1) as spool, \
             tc.tile_pool(name="msgT", bufs=3) as mtpool, \
             tc.tile_pool(name="stile", bufs=6) as stp, \
             tc.tile_pool(name="tmp", bufs=4) as tpool, \
             tc.tile_pool(name="outb", bufs=3) as opool, \
             tc.tile_pool(name="pseg", bufs=4, space="PSUM") as pseg, \
             tc.tile_pool(name="pdns", bufs=3, space="PSUM") as pdns, \
             tc.tile_pool(name="ptrn", bufs=1, space="PSUM") as ptrn:

            # ---- constants ----
            wenc = cpool.tile([IN_DIM, LAT], F32)
            nc.sync.dma_start(out=wenc[:], in_=wenc_d[:])
            benc = cpool.tile([1, LAT], F32)
            nc.sync.dma_start(out=benc[:], in_=benc_d[:])
            ws, wn, bc = [], [], []
            for l in range(2):
                w_ = cpool.tile([LAT, LAT], F16, tag=f"ws{l}")
                nc.sync.dma_start(out=w_[:], in_=ws_d[l][:])
                ws.append(w_)
                w2 = cpool.tile([LAT, LAT], F16, tag=f"wn{l}")
                nc.sync.dma_start(out=w2[:], in_=wn_d[l][:])
                wn.append(w2)
                b_ = cpool.tile([1, LAT], F16, tag=f"bc{l}")
                nc.sync.dma_start(out=b_[:], in_=bc_d[l][:])
                bc.append(b_)
            wout = cpool.tile([LAT, OUT_DIM], F16)
            nc.sync.dma_start(out=wout[:], in_=wout_d[:])
            bout = cpool.tile([1, OUT_DIM], F16)
            nc.sync.dma_start(out=bout[:], in_=bout_d[:])
            invd = cpool.tile([128, NBLK], F32)
            nc.sync.dma_start(out=invd[:], in_=invd_d[:])
            srcrel = cpool.tile([128, NT], F16)
            nc.scalar.dma_start(out=srcrel[:], in_=srcrel_d[:])
            gidx = cpool.tile([128, NT * 8], I16)
            nc.scalar.dma_start(out=gidx[:], in_=gidx_d[:])
            ones16 = cpool.tile([1, 128], F16)
            nc.vector.memset(ones16[:], 1.0)
            ones32 = cpool.tile([1, 128], F32)
            nc.vector.memset(ones32[:], 1.0)
            iota32 = cpool.tile([128, 128], F32)
            nc.gpsimd.iota(iota32[:], [[1, 128]], base=0, channel_multiplier=0,
                           allow_small_or_imprecise_dtypes=True)
            iota = cpool.tile([128, 128], F16)
            nc.vector.tensor_copy(out=iota[:], in_=iota32[:])
            ident = cpool.tile([128, 128], F16)
            make_identity(nc, ident[:])

            mtab = mpool_tab.tile([128, NRANK, 128], F16)
            agg = apool.tile([128, NBLK, 128], F16)
            h_T = hpool.tile([128, NPC], F16)
            mstage = spool.tile([128, NBLK, 128], F16)

            rsem = [nc.alloc_semaphore(f"rsem_s{k}") for k in range(8)]
            lsem = [nc.alloc_semaphore(f"lsem_q{q}") for q in range(4)]
            psem = nc.alloc_semaphore("psem")
            psem2 = nc.alloc_semaphore("psem2")
            ssem = [nc.alloc_semaphore(f"ssem_s{k}") for k in range(8)]
            lsem2 = [nc.alloc_semaphore(f"lsem2_q{q}") for q in range(4)]

            with tc.tile_critical():
                nc.gpsimd.bir_kernel_barrier_wait([[i for i in range(NCORE)]])

            def exchange(e_idx, seg_done_wait, stage_writers):
                """Broadcast mstage into every core's mtab region.  Slot k
                writes region k; data arrives from sender r^DELTA[k].

                No tile_critical: ordering is built entirely from explicit
                dep edges (the broadcast APs are for_isa-lowered and thus
                invisible to Tile's tracker).  Returns (recv_waits,
                send_waits): dependents of the received table / of mstage
                reuse must add_dep on those."""
                with tc.tile_critical():
                    # Pin the critical's placement after the mstage writers
                    # (and whatever seg-done gate applies): criticals float
                    # freely vs same-engine externals unless pre_crit has
                    # deps, and a floated critical that blocks inside while
                    # holding all engine queues deadlocks on hardware.
                    for w in (stage_writers if not NO_PRECRIT_DEPS else []):
                        add_dep_helper(tc.pre_crit_inst, w.ins,
                                       reason="exchange after mstage written")
                    if seg_done_wait is not None:
                        for st in seg_done_wait:
                            add_dep_helper(tc.pre_crit_inst, st,
                                           reason="exchange after all-seg-done")
                    for k in range(NCORE):
                        rdests = [None] * 8
                        rdests[k] = (0, k)
                        nc.gpsimd.remote_dma_broadcast(
                            out_ap=mtab[:, k * NBLK:(k + 1) * NBLK, :].rearrange(
                                "p g f -> p (g f)"),
                            in_ap=mstage[:, :, :].rearrange("p g f -> p (g f)"),
                            remote_sem=rsem[k], local_sem=lsem[k % 4],
                            rdests=rdests, queue_num=k % 4,
                        ).then_inc(psem, 1)
                    if seg_done_wait is not None:
                        # gate: every core finished its seg0 gathers
                        for k in range(NCORE):
                            nc.gpsimd.wait_ge(ssem[k], 2)
                    # explicit-count trigger pattern (HW-proven): wait for
                    # the descriptor writes, then fire 2 preps per queue.
                    nc.gpsimd.wait_ge(psem, 8 * (e_idx + 1))
                    for q in range(4):
                        nc.gpsimd.trigger_dma(count=2, queue_num=q)
                    for k in range(NCORE):
                        nc.sync.wait_ge(rsem[k], 2 * (e_idx + 1))
                    for q in range(4):
                        nc.gpsimd.wait_ge(lsem[q], 32 * (e_idx + 1))
                post = tc.prev_crit_insts[mybir.EngineType.Pool]
                # Make the freshly-received table Tile-visible: tiny self-copy
                # "touch" writes on both int16 windows create a local WRITE on
                # mtab that downstream gather reads (whole-window APs) overlap,
                # so Tile's scheduler sees the real RAW ordering.
                t0 = nc.vector.tensor_copy(out=mtab[0:1, 0, 0:1],
                                           in_=mtab[0:1, 0, 0:1])
                t1 = nc.vector.tensor_copy(out=mtab[0:1, LO_RANKS, 0:1],
                                           in_=mtab[0:1, LO_RANKS, 0:1])
                add_dep_helper(t0.ins, post, reason="touch after exchange")
                add_dep_helper(t1.ins, post, reason="touch after exchange")
                return post

            # ---- encoder ----
            stage_writers = []
            for b in range(NBLK):
                bs = slice(b * 128, (b + 1) * 128)
                fb = tpool.tile([128, 128], F32, tag="fb")
                nc.scalar.dma_start(out=fb[:], in_=featT_d[:, bs])
                p1 = pdns.tile([128, 128], F32, tag="pd")
                nc.tensor.matmul(out=p1[:], lhsT=benc[:], rhs=ones32[:],
                                 start=True, stop=False)
                nc.tensor.matmul(out=p1[:], lhsT=wenc[:], rhs=fb[:],
                                 start=False, stop=True)
                nc.scalar.activation(out=h_T[:, bs], in_=p1[:],
                                     func=mybir.ActivationFunctionType.Copy)
                pm = pdns.tile([128, 128], F32, tag="pd")
                nc.tensor.matmul(out=pm[:], lhsT=h_T[:, bs], rhs=wn[0][:],
                                 start=True, stop=True)
                sw0 = nc.scalar.activation(out=mstage[:, b], in_=pm[:],
                                           func=mybir.ActivationFunctionType.Copy)
                stage_writers.append(sw0)

            with nc.named_scope("exchange0"):
                exch_post = exchange(0, None, stage_writers)

            if DEBUG_MTAB:
                dbg_dma = nc.sync.dma_start(
                    out=dbg_d[:], in_=mtab[:].rearrange("p a b -> p (a b)"))
                add_dep_helper(dbg_dma.ins, exch_post, reason="dbg after exch0")

            # ---- layers ----
            drain_proxies = []   # layer-0 mtab readers: the HBM table dump
            for l in range(2):
                # dump the freshly exchanged SBUF table into HBM, contiguous
                # per partition (row idx' = p*NRANK + rank), so the gathers
                # below read it edge-major with cheap HBM-source descriptors.
                tdump = nc.sync.dma_start(
                    out=table_d[:].rearrange("(p g) f -> p g f", p=128),
                    in_=mtab[:, :, :])
                add_dep_helper(tdump.ins, exch_post,
                               reason="table dump after exchange recv")
                if l == 0:
                    drain_proxies.append(tdump)
                seg_final = []
                with nc.named_scope(f"seg{l}"):
                    cur_psum = None
                    for ci, (t0, nt, w) in enumerate(chunks):
                        msg = mtpool.tile([128, CH_T, 128], F16)
                        src_ap = (table_d[0:LO_ROWS, :] if w == 0
                                  else table_d[LO_ROWS:NTOT, :])
                        nc.gpsimd.dma_gather(
                            out_ap=msg[:, :nt, :], in_ap=src_ap,
                            idxs_ap=gidx[:, t0 * 8:(t0 + nt) * 8],
                            num_idxs=128 * nt, num_idxs_reg=128 * nt,
                            elem_size=LAT, single_packet=False,
                            queue_num=1 + ci % 3)
                        SW = 4   # S tiles built per DVE op
                        sbuilt = {}
                        for j0 in range(0, nt, SW):
                            jn = min(SW, nt - j0)
                            St = stp.tile([128, SW, 128], F16)
                            nc.vector.tensor_tensor(
                                out=St[:, :jn, :],
                                in0=iota[:].unsqueeze(1).to_broadcast([128, jn, 128]),
                                in1=srcrel[:, t0 + j0:t0 + j0 + jn].to_broadcast(
                                    [128, jn, 128]),
                                op=mybir.AluOpType.is_equal)
                            sbuilt[j0] = St
                        for j in range(nt):
                            t = t0 + j
                            b, w_, i = tiles[t]
                            St = sbuilt[j - j % SW]
                            if i == 0:
                                cur_psum = pseg.tile([128, 128], F32, tag="pg")
                            last = (i == T[b, w_] - 1)
                            nc.tensor.matmul(out=cur_psum[:],
                                             lhsT=St[:, j % SW, :],
                                             rhs=msg[:, j, :],
                                             start=(i == 0), stop=last)
                            if last:
                                if w_ == 0:
                                    cp = nc.vector.tensor_copy(out=agg[:, b],
                                                               in_=cur_psum[:])
                                else:
                                    cp = nc.vector.tensor_tensor(
                                        out=agg[:, b], in0=agg[:, b],
                                        in1=cur_psum[:],
                                        op=mybir.AluOpType.add)
                                    seg_final.append(cp)
                # seg-done signal (layer 0 only): +2 to every core's ssem
                # once this core's gathers have fully drained out of mtab
                # (proxied by the last transpose of every chunk).
                ssw_post = None
                if l == 0 and not NO_GATE:
                    ddum = spool.tile([128, NCORE, 8], F16, tag="ddum")
                    with tc.tile_critical():
                        # placement pinned after the last msg copy-back of
                        # every chunk (transitively: after all gather
                        # emissions and their mtab reads); internals are
                        # all Pool, so program order does the rest.
                        for tr in drain_proxies:
                            add_dep_helper(tc.pre_crit_inst, tr.ins,
                                           reason="sd gate after seg drained")
                        for k in range(NCORE):
                            rdests = [None] * 8
                            rdests[k] = (0, k)
                            nc.gpsimd.remote_dma_broadcast(
                                out_ap=ddum[:, k, :],
                                in_ap=ident[:, 0:8],
                                remote_sem=ssem[k], local_sem=lsem2[k % 4],
                                rdests=rdests, queue_num=k % 4,
                            ).then_inc(psem2, 1)
                        nc.gpsimd.wait_ge(psem2, 8)
                        for q in range(4):
                            nc.gpsimd.trigger_dma(count=2, queue_num=q)
                    ssw_post = [tc.prev_crit_insts[mybir.EngineType.Pool]]
                if l == 0:
                    stage_writers = []
                with nc.named_scope(f"dense{l}"):
                    for b in range(NBLK):
                        bs = slice(b * 128, (b + 1) * 128)
                        pd = pdns.tile([128, 128], F32, tag="pd")
                        nc.tensor.matmul(out=pd[:], lhsT=ones16[:], rhs=bc[l][:],
                                         start=True, stop=False)
                        nc.tensor.matmul(out=pd[:], lhsT=h_T[:, bs], rhs=ws[l][:],
                                         start=False, stop=True)
                        tmp = tpool.tile([128, 128], F32, tag="tmp")
                        nc.scalar.activation(out=tmp[:], in_=agg[:, b],
                                             func=mybir.ActivationFunctionType.Copy,
                                             scale=invd[:, b:b + 1])
                        tmp2 = tpool.tile([128, 128], F32, tag="tmp2")
                        nc.vector.tensor_tensor(out=tmp2[:], in0=tmp[:], in1=pd[:],
                                                op=mybir.AluOpType.add)
                        hn = tpool.tile([128, 128], F16, tag="hn")
                        nc.scalar.activation(out=hn[:], in_=tmp2[:],
                                             func=mybir.ActivationFunctionType.Relu)
                        pt = ptrn.tile([128, 128], F16, tag="pt2")
                        nc.tensor.transpose(out=pt[:], in_=hn[:], identity=ident[:])
                        nc.scalar.activation(out=h_T[:, bs], in_=pt[:],
                                             func=mybir.ActivationFunctionType.Copy)
                        if l == 0:
                            pm = pdns.tile([128, 128], F32, tag="pd")
                            nc.tensor.matmul(out=pm[:], lhsT=h_T[:, bs],
                                             rhs=wn[1][:], start=True, stop=True)
                            st_w = nc.scalar.activation(
                                out=mstage[:, b], in_=pm[:],
                                func=mybir.ActivationFunctionType.Copy)
                            add_dep_helper(st_w.ins, exch_post,
                                           reason="mstage reuse after sends")
                            stage_writers.append(st_w)
                if l == 0:
                    with nc.named_scope("exchange1"):
                        exch_post = exchange(1, ssw_post, stage_writers)

            # ---- output ----
            for b in range(NBLK):
                bs = slice(b * 128, (b + 1) * 128)
                po = pdns.tile([128, OUT_DIM], F32, tag="pd")
                nc.tensor.matmul(out=po[:], lhsT=ones16[:], rhs=bout[:],
                                 start=True, stop=False)
                nc.tensor.matmul(out=po[:], lhsT=h_T[:, bs], rhs=wout[:],
                                 start=False, stop=True)
                ob = opool.tile([128, OUT_DIM], F32)
                nc.scalar.activation(out=ob[:], in_=po[:],
                                     func=mybir.ActivationFunctionType.Copy)
                nc.sync.dma_start(out=out_d[bs, :], in_=ob[:])

    nc.compile()
    return nc


def make_in_maps(inputs):
    features = np.asarray(inputs["features"], np.float32)
    edge_list = np.asarray(inputs["edge_list"])
    meta, featT, invd, gidx, srcrel = preprocess(features, edge_list)
    w16 = lambda x: np.asarray(x, np.float16)
    in_maps = []
    for c in range(NCORE):
        in_maps.append(dict(
            featT=featT[c], invd=invd[c], gidx=gidx[c], srcrel=srcrel[c],
            wenc=np.asarray(inputs["W_enc"], np.float32),
            benc=np.asarray(inputs["b_enc"], np.float32).reshape(1, LAT),
            ws0=w16(inputs["W_self"][0]), ws1=w16(inputs["W_self"][1]),
            wn0=w16(inputs["W_neigh"][0]), wn1=w16(inputs["W_neigh"][1]),
            bc0=w16(inputs["b_comb"][0]).reshape(1, LAT),
            bc1=w16(inputs["b_comb"][1]).reshape(1, LAT),
            wout=w16(inputs["W_out"]),
            bout=w16(inputs["b_out"]).reshape(1, OUT_DIM),
        ))
    return meta, in_maps


def assemble(results):
    outs = [results[c]["outp"][:NPC_REAL] for c in range(NCORE)]
    return np.concatenate(outs, axis=0)


def kernel(**inputs):
    """Full-input entry point: shard, compile, run on 8 cores, gather."""
    from concourse import bass_utils
    meta, in_maps = make_in_maps(inputs)
    nc = build(meta)
    res = bass_utils.run_bass_kernel_spmd(nc, in_maps, core_ids=list(range(NCORE)))
    return assemble(res.results)
